# revision 46
# baseline (speedup 1.0000x reference)
"""GCN encoder (2-layer) on 8 Trainium2 NeuronCores.

Math (per layer, matching the reference):
    out[d] = dis[d] * sum_{e: dst_e=d} dis[src_e] * h[src_e]  + b
with h = x @ W, dis = deg^-1/2 over src-with-self-loops. dis factors are
folded host-side: xT is pre-scaled by dis (layer-1 operand), layer-1's
output scaling uses dis^2 (post relu identity: dis*relu(z) = relu(dis*z)),
layer 2 applies dis at the end.

Design notes (measured on this axon-tunneled trn2: SWDGE dma_gather cost
is ~2ns/descriptor across 4 queues with a ~1us/instruction descriptor-gen
component, sub-512B payloads cost the same per descriptor as 512B, and
gathers that hammer one HBM row serialize ~2.6x, so the design minimizes
descriptor count, spreads pad reads, and uses wide multi-packet gathers):
  - dst nodes are assigned to [window, slot] positions sorted by in-degree
    (per core), so the per-window max rank K tracks the degree profile
    instead of the global max. Outputs are unpermuted host-side.
  - edges per dst-window (128 slots) are gathered in [rank, slot] order so
    token k*128+p is the k-th in-edge of window-slot p; one wide
    tensor_reduce over the rank axis aggregates a whole window.
  - THREE overlapping int16-index gather bases (starts 0/8620/17240, 32768
    rows each) instead of two: K0/K2 are pinned at the per-window forced
    maxima and every slot fills streams 0/2 up to those caps, so the
    window rank total tracks max in-degree (~204.8k padded tokens/core vs
    240.6k for the 2-base split). Iterated sort keys (forced counts, then
    effective degree = d + cap shortfalls) cluster the binomial tails.
  - t1 is fp16: halves AllGather-1 wire, SBUF footprint, and DVE reduce
    time (gather descriptor time is unchanged: sub-512B floor). t2 stays
    fp32 (descriptor addresses have 256B granularity, so a 64-col fp16
    row cannot be gathered).
  - rank padding points at the injected all-zero rows (each core ships
    row 6250 zeroed), round-robined across the 5-6 zero rows inside each
    base so no single HBM row is hammered.
  - gathers are 3072-index multi-packet instructions (MAXRANKS=24) on 4
    SWDGE queues, round-robin per instruction; 6-deep token buffers
    decouple the gather WAR from the per-window reduces.
  - each window's self-loop row rides as an extra reduce rank (an aligned
    HWDGE DMA into tok[:, K, :]), so there is no separate own-row tile,
    load, or add.
  - scale/bias/transpose/@W2/output tails are emitted in 8-window blocks
    interleaved into the gather loop (engine streams are in-order, so
    emission position controls overlap); collectives stay after ALL of a
    layer's gathers — a waiting collective would head-of-line block the
    SWDGE stream. Scale ops use stride-0 broadcast APs.
Sharding: nodes row-sharded 6250/core, edges partitioned by dst core,
weights replicated, AllGather between layers.
"""
import os
import numpy as np

N, E = 50000, 1600000
FIN, FHID, FOUT = 256, 128, 64
NCORES = 8
NPC = N // NCORES          # 6250
NPC2 = NPC + 1             # 6251 rows shipped per core (last = zeros)
NFULL = NCORES * NPC2      # 50008
NW = (NPC + 127) // 128    # 49 windows
NPAD = NW * 128            # 6272
HALF = 32768               # rows per int16-index gather base
H2 = NFULL - HALF          # last base start (rows [H2, NFULL))
S1 = H2 // 2               # middle base start (rows [S1, S1 + HALF))
S2 = H2
MAXRANKS = 24              # ranks per gather instruction (3072 idxs,
                           # multi-packet): fewer SWDGE instructions beat
                           # single_packet's ~12%/token edge now that the
                           # per-instruction descriptor-gen serial cost is
                           # a main term (A/B 1.20 vs 1.31ms)

_CACHE = {}
LAST_RESULTS = None


def _host_prep(x, edge_index, W1, b1, W2, b2):
    x = np.asarray(x, dtype=np.float32)
    ei = np.asarray(edge_index)
    W1 = np.asarray(W1, dtype=np.float32)
    W2 = np.asarray(W2, dtype=np.float32)
    b1 = np.asarray(b1, dtype=np.float32)
    b2 = np.asarray(b2, dtype=np.float32)

    loops = np.arange(N, dtype=np.int64)
    src0 = np.concatenate([ei[0].astype(np.int64), loops])
    dst0 = np.concatenate([ei[1].astype(np.int64), loops])

    deg = np.bincount(src0, minlength=N).astype(np.float32)
    dis0 = np.power(deg, np.float32(-0.5), dtype=np.float32)
    dis0[deg == 0] = 0.0

    # Relabel nodes within each core by in-degree (descending) so the
    # per-window max rank K tracks the degree profile instead of the
    # global max: padded gather tokens drop ~40%. Everything downstream
    # (xT, t1, t2, gidx, dis tables) lives in position space; outputs are
    # unpermuted host-side.
    deg_in = np.bincount(dst0, minlength=N)
    srcE0 = ei[0].astype(np.int64)
    dstE0 = ei[1].astype(np.int64)

    def _mk_pos(keys):
        perms, pos_of = [], np.empty(N, np.int64)
        for c in range(NCORES):
            sl0 = slice(c * NPC, (c + 1) * NPC)
            oc = np.lexsort(tuple(k[sl0] for k in keys))
            perms.append(oc)
            pos_of[sl0][oc] = np.arange(NPC)
        return perms, pos_of

    # Three overlapping int16-index gather bases (S0/S1/S2, 32768 rows
    # each) instead of two: nearly every edge has >=2 base choices, so
    # per-slot waterfilling can balance the three stream counts and the
    # window rank count K tracks the max in-degree (sum K ~= sum DegM)
    # instead of paying a lo/hi split penalty (~18% extra tokens).
    _, pos_of = _mk_pos([-deg_in])
    D_d = np.bincount(dstE0, minlength=N)
    wq_all = np.arange(NPC) // 128

    def _cls_of(r):
        return np.where(r < S1, 0,
               np.where(r < S2, 1,
               np.where(r < HALF, 2,
               np.where(r < S1 + HALF, 3, 4))))

    def _dry_n1(pos_of, perms):
        """Per-node stream-1 load of the cap-fill schedule (sort key)."""
        src_p = (srcE0 // NPC) * NPC2 + pos_of[srcE0]
        dstp = (dstE0 // NPC) * NPC + pos_of[dstE0]
        cls = _cls_of(src_p)
        cnt = np.bincount(dstp * 5 + cls, minlength=N * 5).reshape(N, 5)
        wq = (np.arange(N) % NPC) // 128
        K0 = np.zeros(NW, np.int64)
        K2 = np.zeros(NW, np.int64)
        np.maximum.at(K0, wq, cnt[:, 0])
        np.maximum.at(K2, wq, cnt[:, 4])
        cap0 = K0[wq]
        cap2 = K2[wq]
        a01 = np.minimum(cnt[:, 1], np.maximum(0, cap0 - cnt[:, 0]))
        a0f = np.minimum(cnt[:, 2], np.maximum(0, cap0 - cnt[:, 0] - a01))
        a12 = np.minimum(cnt[:, 3], np.maximum(0, cap2 - cnt[:, 4]))
        a2f = np.minimum(cnt[:, 2] - a0f,
                         np.maximum(0, cap2 - cnt[:, 4] - a12))
        # effective degree = d + cap shortfalls: what this slot would force
        # its window's rank total to, if it were the binding slot
        d_all = cnt.sum(axis=1)
        sh0 = np.maximum(0, cap0 - (cnt[:, 0] + cnt[:, 1] + cnt[:, 2]))
        sh2 = np.maximum(0, cap2 - (cnt[:, 4] + cnt[:, 3] +
                                    (cnt[:, 2] - a0f)))
        eff = d_all + sh0 + sh2
        # indexed by global position; map back to node ids
        effN = np.empty(N, np.int64)
        for c in range(NCORES):
            effN[c * NPC + perms[c]] = eff[c * NPC:(c + 1) * NPC]
        return effN

    # iterate: forced-count keys depend on source positions, which the
    # sort itself moves; feeding the previous schedule's stream-1 load
    # back into the keys clusters the flex-shortage tails. A stale key
    # only costs padding, never correctness.
    effN = None
    for _ in range(6):
        rP = (srcE0 // NPC) * NPC2 + pos_of[srcE0]
        m0P = np.bincount(dstE0[rP < S1], minlength=N)
        m2P = np.bincount(dstE0[rP >= S1 + HALF], minlength=N)
        keys = ([-m0P, -m2P, -D_d] if effN is None else
                [-m0P, -m2P, -effN])
        perms, pos_of = _mk_pos(keys)
        effN = _dry_n1(pos_of, perms)
    # Self-loops are excluded from the gather pipeline: the kernel adds each
    # node's own t1/t2 row directly (an aligned DMA load + one add), which
    # removes 6250 real tokens per core per layer and lowers every window's
    # max rank by one. dis still uses the with-loops degree (reference).
    src = (srcE0 // NPC) * NPC + pos_of[srcE0]
    dst = (dstE0 // NPC) * NPC + pos_of[dstE0]
    dis = np.concatenate([dis0[c * NPC:(c + 1) * NPC][perms[c]]
                          for c in range(NCORES)])

    # padded gather row of each source node
    r_all = (src // NPC) * NPC2 + (src % NPC)

    order = np.argsort(dst, kind="stable")
    r_s, dst_s = r_all[order], dst[order]
    cb = np.searchsorted(dst_s, np.arange(NCORES + 1) * NPC)

    # Edge classes by which bases can serve row r:
    #   0: B0 only   1: B0/B1   2: B0/B1/B2   3: B1/B2   4: B2 only
    wq_all = np.arange(NPC) // 128

    def _cls_of(r):
        return np.where(r < S1, 0,
               np.where(r < S2, 1,
               np.where(r < HALF, 2,
               np.where(r < S1 + HALF, 3, 4))))

    # Pass A: K0/K2 are pinned at the per-window forced maxima (edges
    # whose source row is only reachable by base 0 / base 2). Pass B fills
    # every slot's streams 0 and 2 up to those caps and puts the rest in
    # stream 1, so K1 = max(d - n0 - n2): the window total is
    # ~max(DegM, M0M + M2M) = DegM almost everywhere.
    pre = []
    K0a = np.zeros(NW, np.int64)
    K2a = np.zeros(NW, np.int64)
    for c in range(NCORES):
        sl = slice(cb[c], cb[c + 1])
        r_c = r_s[sl]
        d_c = dst_s[sl] - c * NPC
        cls = _cls_of(r_c)
        cnt = np.zeros((5, NPC), np.int64)
        for k in range(5):
            cnt[k] = np.bincount(d_c[cls == k], minlength=NPC)
        np.maximum.at(K0a, wq_all, cnt[0])
        np.maximum.at(K2a, wq_all, cnt[4])
        pre.append((r_c, d_c, cls, cnt))
    K1a = np.zeros(NW, np.int64)
    percore = []
    for c in range(NCORES):
        r_c, d_c, cls, cnt = pre[c]
        cap0 = K0a[wq_all]
        cap2 = K2a[wq_all]
        a01 = np.minimum(cnt[1], np.maximum(0, cap0 - cnt[0]))
        a0f = np.minimum(cnt[2], np.maximum(0, cap0 - cnt[0] - a01))
        a12 = np.minimum(cnt[3], np.maximum(0, cap2 - cnt[4]))
        a2f = np.minimum(cnt[2] - a0f, np.maximum(0, cap2 - cnt[4] - a12))
        n1q = (cnt[1] - a01) + (cnt[2] - a0f - a2f) + (cnt[3] - a12)
        np.maximum.at(K1a, wq_all, n1q)
        # per-edge stream from per-slot class-position thresholds:
        #   c0 -> 0; c4 -> 2; c01: first a01 -> 0 rest -> 1;
        #   c12: first a12 -> 2 rest -> 1;
        #   c012: first a0f -> 0, next a2f -> 2, rest -> 1
        o2 = np.argsort(d_c * 8 + cls, kind="stable")
        d_o, r_o, cls_o = d_c[o2], r_c[o2], cls[o2]
        key_o = d_o * 8 + cls_o
        grp = np.searchsorted(key_o, key_o, side="left")
        i_in = np.arange(len(d_o)) - grp     # index within (slot, class)
        s_o = np.empty(len(d_o), np.int64)
        s_o[cls_o == 0] = 0
        s_o[cls_o == 4] = 2
        m = cls_o == 1
        s_o[m] = np.where(i_in[m] < a01[d_o[m]], 0, 1)
        m = cls_o == 3
        s_o[m] = np.where(i_in[m] < a12[d_o[m]], 2, 1)
        m = cls_o == 2
        s_o[m] = np.where(i_in[m] < a0f[d_o[m]], 0,
                          np.where(i_in[m] < a0f[d_o[m]] + a2f[d_o[m]], 2, 1))
        # rank within (slot, stream)
        o3 = np.argsort(d_o * 4 + s_o, kind="stable")
        d_o, r_o, s_o = d_o[o3], r_o[o3], s_o[o3]
        key3 = d_o * 4 + s_o
        grp3 = np.searchsorted(key3, key3, side="left")
        rank = np.arange(len(d_o)) - grp3
        w_o, p_o = d_o // 128, d_o % 128
        percore.append((w_o, p_o, s_o, rank, r_o))
    # one tensor_reduce per window (reduce instructions are free in this
    # executor), so no pair padding is needed
    K = K0a + K1a + K2a
    # flat token-position offsets: window w = [B0 ranks][B1 ranks][B2 ranks]
    woff = np.zeros(NW + 1, np.int64)
    woff[1:] = np.cumsum(K) * 128
    total_tok = int(woff[-1])

    # all-zero rows (core pads) usable as padding targets, per base;
    # spread pads across them so no single HBM row is hammered
    zrows = np.array([c * NPC2 + NPC for c in range(NCORES)], np.int64)
    zpads = [zrows[(zrows >= s) & (zrows < s + HALF)] - s
             for s in (0, S1, S2)]
    in_maps = []
    for c in range(NCORES):
        w_o, p_o, s_o, rank, r_o = percore[c]
        gidx = np.empty(total_tok, np.int16)
        for w in range(NW):
            q0 = woff[w]
            q1 = q0 + K0a[w] * 128
            q2 = q1 + K1a[w] * 128
            q3 = woff[w + 1]
            for (a, b, zp) in ((q0, q1, zpads[0]), (q1, q2, zpads[1]),
                               (q2, q3, zpads[2])):
                if b > a:
                    gidx[a:b] = zp[np.arange(b - a) % len(zp)]
        sbase = np.where(s_o == 0, 0, np.where(s_o == 1, S1, S2))
        sskip = np.where(s_o == 0, 0,
                         np.where(s_o == 1, K0a[w_o], K0a[w_o] + K1a[w_o]))
        pos = woff[w_o] + (rank + sskip) * 128 + p_o
        gidx[pos] = (r_o - sbase).astype(np.int16)
        gidx_t = np.tile(gidx.reshape(-1, 16).T, (8, 1))  # [128, total_tok//16]

        dis_l = dis[c * NPC:(c + 1) * NPC]     # position space
        dis_pad = np.zeros(NPAD, np.float32)
        dis_pad[:NPC] = dis_l
        dis_col = np.ascontiguousarray(dis_pad.reshape(NW, 128).T)  # [128, NW]
        dis2_col = dis_col * dis_col
        # Bstt[p, w*128+f] = dis[w*128+p] * b1[f]
        Bstt = (dis_col.T[:, :, None] * b1[None, None, :]).transpose(1, 0, 2)
        Bstt = np.ascontiguousarray(Bstt.reshape(128, NW * FHID))

        xT = np.zeros((FIN, NPAD), np.float32)
        xT[:, :NPC] = (x[c * NPC + perms[c]] * dis_l[:, None]).T

        in_maps.append({
            "gidx": np.ascontiguousarray(gidx_t),
            "xT": xT,
            "W1": W1, "W2": W2,
            "dis2c": dis2_col, "disc": dis_col,
            "Bstt": Bstt,
            "b2b": np.tile(b2, (128, 1)),
            "ident": np.eye(128, dtype=np.float32),
        })
    return in_maps, (K0a, K1a, K2a,
                     bool(not b1.any()), bool(not b2.any())), perms


def _build(Kinfo):
    import concourse.bacc as bacc
    import concourse.mybir as mybir
    import concourse.tile as tile

    K0a, K1a, K2a, B1ZERO, B2ZERO = Kinfo
    K = K0a + K1a + K2a
    maxK = int(K.max())
    total_tok = int(K.sum()) * 128

    PHASES = os.environ.get("GCN_PHASES", "full")
    REPEAT = int(os.environ.get("GCN_REPEAT", "1"))
    SKIPGATHER = bool(os.environ.get("GCN_SKIPGATHER"))  # timing probe only
    SKIPREDUCE = bool(os.environ.get("GCN_SKIPREDUCE"))  # timing probe only
    SKIPAG = bool(os.environ.get("GCN_SKIPAG"))          # timing probe only
    MR = int(os.environ.get("GCN_MAXRANKS", str(MAXRANKS)))
    NQ = int(os.environ.get("GCN_QUEUES", "4"))  # spread gathers over 4 SWDGE queues: -19% (A/B 3.47 vs 4.30ms)
    T1F16 = os.environ.get("GCN_T1F16", "1") == "1"  # t1 in fp16: halves L1 gather + AG1 bytes

    dt = mybir.dt
    ALU = mybir.AluOpType
    t1dt = dt.float16 if T1F16 else dt.float32

    nc = bacc.Bacc("TRN2", target_bir_lowering=False, debug=False,
                   num_devices=NCORES,
                   **({"num_swdge_queues": NQ} if NQ > 1 else {}))

    gidx_d = nc.dram_tensor("gidx", [128, total_tok // 16], dt.int16, kind="ExternalInput")
    xT_d = nc.dram_tensor("xT", [FIN, NPAD], dt.float32, kind="ExternalInput")
    W1_d = nc.dram_tensor("W1", [FIN, FHID], dt.float32, kind="ExternalInput")
    W2_d = nc.dram_tensor("W2", [FHID, FOUT], dt.float32, kind="ExternalInput")
    dis2_d = nc.dram_tensor("dis2c", [128, NW], dt.float32, kind="ExternalInput")
    dis_d = nc.dram_tensor("disc", [128, NW], dt.float32, kind="ExternalInput")
    Bstt_d = nc.dram_tensor("Bstt", [128, NW * FHID], dt.float32, kind="ExternalInput")
    b2b_d = nc.dram_tensor("b2b", [128, FOUT], dt.float32, kind="ExternalInput")
    ident_d = nc.dram_tensor("ident", [128, 128], dt.float32, kind="ExternalInput")
    out_d = nc.dram_tensor("out", [NPC, FOUT], dt.float32, kind="ExternalOutput")

    # double-buffered across repeats: repeat r+1's phase B / AllGather can
    # overlap repeat r's L2 gathers instead of serializing on a WAR hazard
    t1_locals = [nc.dram_tensor(f"t1_local{i}", [NPC2, FHID], t1dt)
                 for i in range(2)]
    t1_fulls = [nc.dram_tensor(f"t1_full{i}", [NFULL, FHID], t1dt,
                               addr_space="Shared") for i in range(2)]
    t2_locals = [nc.dram_tensor(f"t2_local{i}", [NPC2, FOUT], dt.float32)
                 for i in range(2)]
    t2_fulls = [nc.dram_tensor(f"t2_full{i}", [NFULL, FOUT], dt.float32,
                               addr_space="Shared") for i in range(2)]

    NWF = NW - 1  # 48 full windows; window 48 has 106 live rows

    with tile.TileContext(nc) as tc:
        with (
            tc.tile_pool(name="consts", bufs=1) as cp,
            tc.tile_pool(name="work", bufs=1) as wp,
            tc.tile_pool(name="psum", bufs=1, space="PSUM") as pp,
        ):
            ident_t = cp.tile([128, 128], dt.float32, tag="ident")
            nc.sync.dma_start(ident_t[:], ident_d[:, :])
            w1_t = cp.tile([128, 2, FHID], dt.float32, tag="w1")
            nc.sync.dma_start(w1_t[:, 0, :], W1_d[0:128, :])
            nc.sync.dma_start(w1_t[:, 1, :], W1_d[128:256, :])
            w2_t = cp.tile([FHID, FOUT], dt.float32, tag="w2")
            nc.sync.dma_start(w2_t[:], W2_d[:, :])
            dis2_t = cp.tile([128, NW], dt.float32, tag="dis2")
            nc.sync.dma_start(dis2_t[:], dis2_d[:, :])
            dis_t = cp.tile([128, NW], dt.float32, tag="dis")
            nc.sync.dma_start(dis_t[:], dis_d[:, :])
            if not B1ZERO:
                Bstt_t = cp.tile([128, NW * FHID], dt.float32, tag="Bstt")
                nc.sync.dma_start(Bstt_t[:], Bstt_d[:, :])
            b2b_t = cp.tile([128, FOUT], dt.float32, tag="b2b")
            nc.sync.dma_start(b2b_t[:], b2b_d[:, :])
            gidx_t = cp.tile([128, total_tok // 16], dt.int16, tag="gidx")
            nc.sync.dma_start(gidx_t[:], gidx_d[:, :])
            zrow = cp.tile([128, FHID], dt.float32, tag="zrow")
            nc.vector.memset(zrow[:], 0.0)
            zrow1 = zrow
            if T1F16:
                zrow1 = cp.tile([128, FHID], t1dt, tag="zrow1")
                nc.vector.memset(zrow1[:], 0.0)

            # one shared gpsimd register per distinct gather count: avoids a
            # RegisterMove instruction (~55us here) per dma_gather
            counts = set()
            for w in range(NW):
                for nk in (int(K0a[w]), int(K1a[w]), int(K2a[w])):
                    for k0 in range(0, nk, MR):
                        counts.add(min(MR, nk - k0) * 128)
            counts.discard(0)
            nidx_regs = {cnt: nc.gpsimd.to_reg(cnt) for cnt in sorted(counts)}

            dis2_bc = dis2_t[:].rearrange("p (w o) -> p w o", o=1) \
                               .broadcast_to([128, NW, FHID])
            dis_bc = dis_t[:].rearrange("p (w o) -> p w o", o=1) \
                             .broadcast_to([128, NW, FOUT])
            b2_bc = b2b_t[:].rearrange("(o p) f -> p o f", o=1) \
                            .broadcast_to([128, NW, FOUT])

            for _rep in range(REPEAT):
                t1_local, t1_full = t1_locals[_rep % 2], t1_fulls[_rep % 2]
                t2_local, t2_full = t2_locals[_rep % 2], t2_fulls[_rep % 2]
                # ---- phase B: t1_local = (dis*x) @ W1 ----
                with tc.tile_pool(name="phaseB", bufs=1) as pb:
                    xT_t = pb.tile([128, 2, NPAD], dt.float32, tag="xT")
                    nc.sync.dma_start(xT_t[:, 0, :], xT_d[0:128, :])
                    nc.sync.dma_start(xT_t[:, 1, :], xT_d[128:256, :])
                    evB = pb.tile([128, NWF, FHID], t1dt, tag="evB")
                    evBt = pb.tile([128, FHID], t1dt, tag="evBt")
                    psB = pp.tile([128, 8, FHID], dt.float32, tag="pB")
                    for w in range(NW):
                        sl = psB[:, w % 8, :]
                        for kc in range(2):
                            nc.tensor.matmul(
                                sl, xT_t[:, kc, w * 128:w * 128 + 128],
                                w1_t[:, kc, :], start=(kc == 0), stop=(kc == 1))
                        if w % 8 == 7:
                            nc.vector.tensor_copy(evB[:, w - 7:w + 1, :], psB[:])
                            # stream each 8-window chunk to DRAM as it
                            # lands so AllGather-1 starts on a short tail
                            nc.sync.dma_start(
                                t1_local[(w - 7) * 128:(w + 1) * 128, :]
                                .rearrange("(a p) f -> p a f", p=128),
                                evB[:, w - 7:w + 1, :])
                        if w == NW - 1:
                            nc.vector.tensor_copy(evBt[:], sl)
                    nc.sync.dma_start(t1_local[NWF * 128:NPC, :],
                                      evBt[0:NPC - NWF * 128, :])
                    nc.sync.dma_start(t1_local[NPC:NPC2, :], zrow1[0:1, :])

                if SKIPAG:
                    nc.sync.dma_start(t1_full[0:NPC2, :], t1_local[:, :])
                else:
                    nc.gpsimd.collective_compute(
                        "AllGather", mybir.AluOpType.bypass,
                        replica_groups=[list(range(NCORES))],
                        ins=[t1_local[:, :]], outs=[t1_full[:, :]],
                    )

                if PHASES == "Bdump":
                    # debug: out <- first FOUT cols of t1_local
                    nc.sync.dma_start(out_d[:, :], t1_local[0:NPC, 0:FOUT])
                    continue

                if PHASES == "B":
                    ot = wp.tile([128, FOUT], dt.float32, tag="o")
                    nc.vector.memset(ot[:], 0.0)
                    for w in range(NW):
                        rows = min(128, NPC - w * 128)
                        nc.sync.dma_start(out_d[w * 128:w * 128 + rows, :],
                                          ot[0:rows, :])
                    continue

                qrr = [0]  # per-gather round-robin queue counter

                def gather_window(tok, w, src_full, feat, woff_w, dk=0):
                    """Emit gathers for window w into tok at rank offset dk."""
                    k0, k1, k2 = int(K0a[w]), int(K1a[w]), int(K2a[w])
                    segs = [(0, k0, src_full[0:HALF, :]),
                            (k0, k1, src_full[S1:S1 + HALF, :]),
                            (k0 + k1, k2, src_full[S2:NFULL, :])]
                    for seg0, nk, base in segs:
                        for k0 in range(0, nk, MR):
                            kn = min(MR, nk - k0)
                            c0 = (woff_w + (seg0 + k0) * 128) // 16
                            d0 = dk + seg0 + k0
                            if SKIPGATHER:
                                continue
                            nc.gpsimd.dma_gather(
                                tok[:, d0:d0 + kn, :], base,
                                gidx_t[:, c0:c0 + kn * 8],
                                num_idxs=kn * 128,
                                num_idxs_reg=nidx_regs[kn * 128],
                                elem_size=feat,
                                single_packet=(kn * 128 <= 1024),
                                queue_num=(qrr[0] % NQ
                                           if os.environ.get("GCN_QRR", "1") == "1"
                                           else w % NQ))
                            qrr[0] += 1

                # ---- L1 pass 1: gather + reduce into red_all, one fused
                #      relu+scale over all windows ----
                with tc.tile_pool(name="L1", bufs=1) as l1:
                    NTOK = int(os.environ.get("GCN_NTOK", "6"))
                    toks = [l1.tile([128, maxK + 1, FHID], t1dt, tag=f"tok1{i}",
                                    name=f"tok1{i}") for i in range(NTOK)]
                    red_all = l1.tile([128, NW, FHID], dt.float32, tag="redA")
                    o1s_all = l1.tile([128, NW, FHID], dt.float32, tag="o1sa")
                    o1T = l1.tile([128, 4, FHID], dt.float32, tag="o1T")
                    ev1 = l1.tile([128, NWF, FOUT], dt.float32, tag="ev1")
                    ev1t = l1.tile([128, FOUT], dt.float32, tag="ev1t")
                    pT = pp.tile([128, 4, 512], dt.float32, tag="pT")  # slice per bank
                    p2 = pp.tile([128, 8, FOUT], dt.float32, tag="p2")
                    if SKIPGATHER:
                        for t in toks:
                            nc.vector.memset(t[:], 0.0)
                    if SKIPREDUCE:
                        nc.vector.memset(red_all[:], 0.0)

                    def emit_half1(wa, wb):
                        """own-add + relu-scale + transpose@W2 + t2_local DMA
                        for windows [wa, wb). Emitted right after window
                        wb-1's reduce so the DVE/tensor work fills gather
                        idle time; the collectives are emitted separately
                        after ALL gathers (in-order SWDGE stream: a waiting
                        collective would head-of-line block the gathers)."""
                        sl = slice(wa, wb)
                        dis2_h = dis2_t[:, sl].rearrange(
                            "p (w o) -> p w o", o=1) \
                            .broadcast_to([128, wb - wa, FHID])
                        # o1s = relu(dis^2*red + dis*b1)
                        if B1ZERO:
                            nc.vector.scalar_tensor_tensor(
                                o1s_all[:, sl, :], red_all[:, sl, :], 0.0,
                                dis2_h, ALU.max, ALU.mult)
                        else:
                            nc.vector.scalar_tensor_tensor(
                                o1s_all[:, sl, :], red_all[:, sl, :], 0.0,
                                dis2_h, ALU.bypass, ALU.mult)
                            nc.vector.scalar_tensor_tensor(
                                o1s_all[:, sl, :], o1s_all[:, sl, :], 0.0,
                                Bstt_t[:].rearrange("p (w f) -> p w f",
                                                    w=NW)[:, sl, :],
                                ALU.bypass, ALU.add)
                            nc.vector.tensor_scalar(
                                o1s_all[:, sl, :], o1s_all[:, sl, :], 0.0,
                                None, ALU.max)
                        # pass 2: transpose + @W2 (group logic needs
                        # HW1 % 8 == 0)
                        for w in range(wa, wb):
                            nc.tensor.transpose(pT[:, w % 4, 0:FHID],
                                                o1s_all[:, w, :], ident_t[:])
                            if w % 4 == 3:
                                nc.vector.tensor_copy(o1T[:], pT[:, :, 0:FHID])
                            if w == NW - 1:
                                nc.vector.tensor_copy(o1T[:, 0, :],
                                                      pT[:, 0, 0:FHID])
                            if w % 4 == 3 or w == NW - 1:
                                for w2 in range(
                                        w - (3 if w % 4 == 3 else 0), w + 1):
                                    nc.tensor.matmul(p2[:, w2 % 8, :],
                                                     o1T[:, w2 % 4, :], w2_t[:],
                                                     start=True, stop=True)
                            if w % 8 == 7:
                                nc.vector.tensor_copy(ev1[:, w - 7:w + 1, :],
                                                      p2[:])
                            if w == NW - 1:
                                nc.vector.tensor_copy(ev1t[:], p2[:, 0, :])
                        if wb < NW:
                            nc.sync.dma_start(
                                t2_local[wa * 128:wb * 128, :]
                                .rearrange("(a p) f -> p a f", p=128),
                                ev1[:, wa:wb, :])
                        else:
                            if NWF > wa:
                                nc.sync.dma_start(
                                    t2_local[wa * 128:NWF * 128, :]
                                    .rearrange("(a p) f -> p a f", p=128),
                                    ev1[:, wa:NWF, :])
                            nc.sync.dma_start(t2_local[NWF * 128:NPC, :],
                                              ev1t[0:NPC - NWF * 128, :])
                            nc.sync.dma_start(t2_local[NPC:NPC2, :],
                                              zrow[0:1, 0:FOUT])

                    HWB = int(os.environ.get("GCN_HWB", "8"))
                    HWS = list(range(HWB, NW, HWB)) + [NW]  # block ends
                    woff_w = 0
                    prev = 0
                    for w in range(NW):
                        tok = toks[w % NTOK]
                        gather_window(tok, w, t1_full, FHID, woff_w, 0)
                        woff_w += int(K[w]) * 128
                        kw = int(K[w])
                        rows = min(128, NPC - w * 128)
                        if rows < 128:
                            nc.vector.memset(tok[:, kw:kw + 1, :], 0.0)
                        nc.sync.dma_start(
                            tok[0:rows, kw:kw + 1, :],
                            t1_local[w * 128:w * 128 + rows, :]
                            .rearrange("(a p) f -> p a f", a=1))
                        if not SKIPREDUCE:
                            nc.vector.tensor_reduce(
                                red_all[:, w:w + 1, :],
                                tok[:, 0:kw + 1, :]
                                .rearrange("p (b k) f -> p b f k", b=1),
                                mybir.AxisListType.X, ALU.add)
                        if w + 1 in HWS and w + 1 < NW:
                            emit_half1(prev, w + 1)
                            prev = w + 1
                    emit_half1(prev, NW)
                    if not SKIPAG:
                        nc.gpsimd.collective_compute(
                            "AllGather", mybir.AluOpType.bypass,
                            replica_groups=[list(range(NCORES))],
                            ins=[t2_local[:, :]], outs=[t2_full[:, :]],
                        )

                if PHASES == "B1dump":
                    nc.sync.dma_start(out_d[:, :], t2_local[0:NPC, :])
                    continue

                if PHASES == "B1":
                    ot = wp.tile([128, FOUT], dt.float32, tag="o")
                    nc.vector.memset(ot[:], 0.0)
                    for w in range(NW):
                        rows = min(128, NPC - w * 128)
                        nc.sync.dma_start(out_d[w * 128:w * 128 + rows, :],
                                          ot[0:rows, :])
                    continue

                if SKIPAG:
                    nc.sync.dma_start(t2_full[0:NPC2, :], t2_local[:, :])

                # ---- L2 windows ----
                with tc.tile_pool(name="L2", bufs=1) as l2:
                    tok2s = [l2.tile([128, maxK + 1, FOUT], dt.float32, tag=f"tok2{i}",
                                     name=f"tok2{i}") for i in range(NTOK)]
                    red2_all = l2.tile([128, NW, FOUT], dt.float32, tag="red2A")
                    ev2_all = l2.tile([128, NW, FOUT], dt.float32, tag="ev2A")
                    if SKIPGATHER:
                        for t in tok2s:
                            nc.vector.memset(t[:], 0.0)
                    if SKIPREDUCE:
                        nc.vector.memset(red2_all[:], 0.0)

                    def emit_half2(wa, wb):
                        """own-add + dis scale (+ b2) + out DMA for windows
                        [wa, wb), interleaved into the gather loop so the
                        DVE work fills gather idle time."""
                        sl = slice(wa, wb)
                        dis_h = dis_t[:, sl].rearrange(
                            "p (w o) -> p w o", o=1) \
                            .broadcast_to([128, wb - wa, FOUT])
                        nc.vector.scalar_tensor_tensor(
                            ev2_all[:, sl, :], red2_all[:, sl, :], 0.0,
                            dis_h, ALU.bypass, ALU.mult)
                        if not B2ZERO:
                            nc.vector.scalar_tensor_tensor(
                                ev2_all[:, sl, :], ev2_all[:, sl, :], 0.0,
                                b2_bc[:, sl, :], ALU.bypass, ALU.add)
                        if wb < NW:
                            nc.sync.dma_start(
                                out_d[wa * 128:wb * 128, :]
                                .rearrange("(a p) f -> p a f", p=128),
                                ev2_all[:, wa:wb, :])
                        else:
                            if NWF > wa:
                                nc.sync.dma_start(
                                    out_d[wa * 128:NWF * 128, :]
                                    .rearrange("(a p) f -> p a f", p=128),
                                    ev2_all[:, wa:NWF, :])
                            nc.sync.dma_start(
                                out_d[NWF * 128:NPC, :],
                                ev2_all[0:NPC - NWF * 128, NWF, :])

                    woff_w = 0
                    prev = 0
                    for w in range(NW):
                        tok2 = tok2s[w % NTOK]
                        gather_window(tok2, w, t2_full, FOUT, woff_w, 0)
                        woff_w += int(K[w]) * 128
                        kw = int(K[w])
                        rows = min(128, NPC - w * 128)
                        if rows < 128:
                            nc.vector.memset(tok2[:, kw:kw + 1, :], 0.0)
                        nc.sync.dma_start(
                            tok2[0:rows, kw:kw + 1, :],
                            t2_local[w * 128:w * 128 + rows, :]
                            .rearrange("(a p) f -> p a f", a=1))
                        if not SKIPREDUCE:
                            nc.vector.tensor_reduce(
                                red2_all[:, w:w + 1, :],
                                tok2[:, 0:kw + 1, :]
                                .rearrange("p (b k) f -> p b f k", b=1),
                                mybir.AxisListType.X, ALU.add)
                        if w + 1 in HWS and w + 1 < NW:
                            emit_half2(prev, w + 1)
                            prev = w + 1
                    emit_half2(prev, NW)

    nc.compile()
    return nc


def kernel(x, edge_index, W1, b1, W2, b2):
    global LAST_RESULTS
    from concourse.bass_utils import run_bass_kernel_spmd

    in_maps, Kinfo, perms = _host_prep(x, edge_index, W1, b1, W2, b2)
    key = (Kinfo[0].tobytes(), Kinfo[1].tobytes(), Kinfo[2].tobytes(),
           Kinfo[3], Kinfo[4])
    if key not in _CACHE:
        _CACHE[key] = _build(Kinfo)
    nc = _CACHE[key]

    res = run_bass_kernel_spmd(nc, in_maps, list(range(NCORES)))
    LAST_RESULTS = res
    out = np.empty((N, FOUT), np.float32)
    for c in range(NCORES):
        out[c * NPC + perms[c]] = res.results[c]["out"]
    return out

